# revision 1
# baseline (speedup 1.0000x reference)
"""Trainium2 Bass kernel for a fused GRUCell step.

Math (reference):
    xi = x @ [W_ir W_iz W_in] + [b_ir b_iz b_in]
    hh = h @ [W_hr W_hz W_hn]
    r = sigmoid(xr + hr); z = sigmoid(xz + hz)
    n = tanh(xn + r * (hn + b_hn))
    new_h = (1 - z) * n + z * h

Strategy: pure data-parallel over the batch dim (B=16384 -> 8 cores x 2048).
Weights are replicated. Per core, one K-concatenated GEMM family with
K = F + H = 2048: lhsT = [x_shard; h_shard]^T (fp16), rhs = per-gate
[W_i*; W_h*] concat (fp16). The r and z gates accumulate x- and h-products
into the same PSUM bank (K=2048); the n gate keeps xn and hn separate
(the recurrence multiplies hn by r before adding xn). Elementwise gates run
on ScalarE (sigmoid/tanh) + VectorE (mul/add/sub), fp32 throughout.

fp16 matmul runs at 1 cycle/row on the PE (fp32 would be 2) and keeps
~11-bit mantissas; accumulation is fp32 in PSUM.
"""

import os
import sys

import numpy as np

sys.path.insert(0, "/opt/trn_rl_repo")
os.environ.setdefault("MYCRO_LOCAL_CACHE", "1")

import concourse.bass as bass  # noqa: E402
import concourse.mybir as mybir  # noqa: E402
import concourse.tile as tile  # noqa: E402
from concourse import bacc  # noqa: E402
from concourse.bass_utils import run_bass_kernel_spmd  # noqa: E402

N_CORES = 8
F = 1024  # input feature dim
H = 1024  # hidden dim
K = F + H  # GEMM contraction dim (x features then h features)
P = 128
KO = K // P  # 16 k-chunks of 128
KOX = F // P  # 8 k-chunks belonging to the x part
MBLK = 512  # batch rows staged per lhsT DMA block
NC_CHUNK = 512  # H columns per PSUM bank / matmul


def build_gru_program(b_core: int, with_bias: bool) -> bass.Bass:
    """One SPMD program; every core runs it on its own batch shard."""
    fp16 = mybir.dt.float16
    f32 = mybir.dt.float32
    n_blk = b_core // MBLK
    assert b_core % MBLK == 0
    hc_n = H // NC_CHUNK

    # Bacc (not plain Bass): its compile pipeline splits multi-sem waits into
    # event semaphores — walrus rejects >1 wait on most engine instructions.
    nc = bacc.Bacc()
    lhsT = nc.declare_dram_parameter("lhsT", [n_blk, K, MBLK], fp16, isOutput=False)
    wr = nc.declare_dram_parameter("wr", [K, H], fp16, isOutput=False)
    wz = nc.declare_dram_parameter("wz", [K, H], fp16, isOutput=False)
    wn = nc.declare_dram_parameter("wn", [K, H], fp16, isOutput=False)
    h32 = nc.declare_dram_parameter("h32", [b_core, H], f32, isOutput=False)
    if with_bias:
        # host-replicated across partitions; rows: b_r, b_z, b_in, b_hn
        bias = nc.declare_dram_parameter("bias_rep", [P, 4, H], f32, isOutput=False)
    out = nc.declare_dram_parameter("out", [b_core, H], f32, isOutput=True)

    Sigmoid = mybir.ActivationFunctionType.Sigmoid
    Tanh = mybir.ActivationFunctionType.Tanh

    with tile.TileContext(nc) as tc:
        with (
            tc.tile_pool(name="wpool", bufs=1) as wpool,
            tc.tile_pool(name="lpool", bufs=2) as lpool,
            tc.tile_pool(name="hpool", bufs=3) as hpool,
            tc.tile_pool(name="opool", bufs=3) as opool,
            tc.tile_pool(name="epool", bufs=2 if with_bias else 3) as epool,
            tc.tile_pool(name="psum", bufs=2, space="PSUM") as psum,
        ):
            # Resident weights, split per H-chunk so the first chunk's matmuls
            # can start before all 12.6MB of weights have landed.
            wsb = {}
            for hc in range(hc_n):
                cs = slice(hc * NC_CHUNK, (hc + 1) * NC_CHUNK)
                for gname, wparam in (("r", wr), ("z", wz), ("n", wn)):
                    t = wpool.tile([P, KO, NC_CHUNK], fp16, tag=f"w{gname}{hc}")
                    nc.sync.dma_start(
                        t[:],
                        wparam[:].rearrange("(ko p) n -> p ko n", p=P)[:, :, cs],
                    )
                    wsb[(gname, hc)] = t

            bias_sb = None
            if with_bias:
                bias_sb = wpool.tile([P, 4, H], f32, tag="bias_sb")
                nc.sync.dma_start(bias_sb[:], bias[:])

            for blk in range(n_blk):
                lt = lpool.tile([P, KO, MBLK], fp16, tag="lhsT")
                nc.sync.dma_start(
                    lt[:], lhsT[blk].rearrange("(ko p) m -> p ko m", p=P)
                )
                for ms in range(MBLK // P):
                    m0 = blk * MBLK + ms * P
                    ht = hpool.tile([P, H], f32, tag="hnat")
                    nc.sync.dma_start(ht[:], h32[m0 : m0 + P, :])
                    ot = opool.tile([P, H], f32, tag="out")
                    for hc in range(hc_n):
                        cs = slice(hc * NC_CHUNK, (hc + 1) * NC_CHUNK)
                        pr = psum.tile([P, NC_CHUNK], f32, tag="pr")
                        pz = psum.tile([P, NC_CHUNK], f32, tag="pz")
                        pxn = psum.tile([P, NC_CHUNK], f32, tag="pxn")
                        phn = psum.tile([P, NC_CHUNK], f32, tag="phn")
                        for ko in range(KO):
                            ls = lt[:, ko, ms * P : (ms + 1) * P]
                            nc.tensor.matmul(
                                pr[:],
                                ls,
                                wsb[("r", hc)][:, ko, :],
                                start=(ko == 0),
                                stop=(ko == KO - 1),
                            )
                            nc.tensor.matmul(
                                pz[:],
                                ls,
                                wsb[("z", hc)][:, ko, :],
                                start=(ko == 0),
                                stop=(ko == KO - 1),
                            )
                            if ko < KOX:
                                nc.tensor.matmul(
                                    pxn[:],
                                    ls,
                                    wsb[("n", hc)][:, ko, :],
                                    start=(ko == 0),
                                    stop=(ko == KOX - 1),
                                )
                            else:
                                nc.tensor.matmul(
                                    phn[:],
                                    ls,
                                    wsb[("n", hc)][:, ko, :],
                                    start=(ko == KOX),
                                    stop=(ko == KO - 1),
                                )

                        sr = epool.tile([P, NC_CHUNK], f32, tag="sr")
                        sz = epool.tile([P, NC_CHUNK], f32, tag="sz")
                        sn = epool.tile([P, NC_CHUNK], f32, tag="sn")
                        tt = epool.tile([P, NC_CHUNK], f32, tag="tt")
                        if with_bias:
                            nc.vector.tensor_add(tt[:], pr[:], bias_sb[:, 0, cs])
                            nc.scalar.activation(sr[:], tt[:], Sigmoid)
                            nc.vector.tensor_add(tt[:], pz[:], bias_sb[:, 1, cs])
                            nc.scalar.activation(sz[:], tt[:], Sigmoid)
                            nc.vector.tensor_add(tt[:], phn[:], bias_sb[:, 3, cs])
                            nc.vector.tensor_mul(tt[:], sr[:], tt[:])
                            nc.vector.tensor_add(tt[:], tt[:], pxn[:])
                            nc.vector.tensor_add(tt[:], tt[:], bias_sb[:, 2, cs])
                        else:
                            nc.scalar.activation(sr[:], pr[:], Sigmoid)
                            nc.scalar.activation(sz[:], pz[:], Sigmoid)
                            nc.vector.tensor_mul(tt[:], sr[:], phn[:])
                            nc.vector.tensor_add(tt[:], tt[:], pxn[:])
                        nc.scalar.activation(sn[:], tt[:], Tanh)
                        nc.vector.tensor_sub(tt[:], ht[:, cs], sn[:])
                        nc.vector.tensor_mul(tt[:], tt[:], sz[:])
                        nc.vector.tensor_add(ot[:, cs], sn[:], tt[:])
                    nc.sync.dma_start(out[m0 : m0 + P, :], ot[:])
    nc.finalize()
    return nc


_PROGRAM_CACHE: dict = {}


def get_program(b_core: int, with_bias: bool) -> bass.Bass:
    key = (b_core, with_bias)
    if key not in _PROGRAM_CACHE:
        _PROGRAM_CACHE[key] = build_gru_program(b_core, with_bias)
    return _PROGRAM_CACHE[key]


def prepare_in_maps(h, x, W_ir, W_iz, W_in, b_ir, b_iz, b_in, W_hr, W_hz, W_hn, b_hn):
    """Host-side shard + layout prep. Returns (in_maps, with_bias, b_core)."""
    h = np.ascontiguousarray(np.asarray(h, dtype=np.float32))
    x = np.ascontiguousarray(np.asarray(x, dtype=np.float32))
    b_full = x.shape[0]
    assert b_full % N_CORES == 0
    b_core = b_full // N_CORES
    n_blk = b_core // MBLK

    wr_ = np.concatenate([W_ir, W_hr], axis=0).astype(np.float16)
    wz_ = np.concatenate([W_iz, W_hz], axis=0).astype(np.float16)
    wn_ = np.concatenate([W_in, W_hn], axis=0).astype(np.float16)

    br = np.asarray(b_ir, np.float32)
    bz = np.asarray(b_iz, np.float32)
    bn = np.asarray(b_in, np.float32)
    bhn = np.asarray(b_hn, np.float32)
    biases = np.stack([br, bz, bn, bhn]).astype(np.float32)
    with_bias = bool(np.any(biases != 0.0))

    in_maps = []
    for c in range(N_CORES):
        sl = slice(c * b_core, (c + 1) * b_core)
        xc = x[sl]
        hc = h[sl]
        lhsT_full = np.empty((K, b_core), np.float16)
        lhsT_full[:F] = xc.T
        lhsT_full[F:] = hc.T
        lhsT_t = np.ascontiguousarray(
            lhsT_full.reshape(K, n_blk, MBLK).transpose(1, 0, 2)
        )
        m = {
            "lhsT": lhsT_t,
            "wr": wr_,
            "wz": wz_,
            "wn": wn_,
            "h32": np.ascontiguousarray(hc),
        }
        if with_bias:
            m["bias_rep"] = np.ascontiguousarray(
                np.broadcast_to(biases[None], (P, 4, H))
            )
        in_maps.append(m)
    return in_maps, with_bias, b_core


def kernel(h, x, W_ir, W_iz, W_in, b_ir, b_iz, b_in, W_hr, W_hz, W_hn, b_hn):
    in_maps, with_bias, b_core = prepare_in_maps(
        h, x, W_ir, W_iz, W_in, b_ir, b_iz, b_in, W_hr, W_hz, W_hn, b_hn
    )
    nc = get_program(b_core, with_bias)
    res = run_bass_kernel_spmd(nc, in_maps, list(range(N_CORES)))
    new_h = np.concatenate([res.results[c]["out"] for c in range(N_CORES)], axis=0)
    return (new_h, new_h)



# revision 2
# speedup vs baseline: 1.1050x; 1.1050x over previous
"""Trainium2 Bass kernel for a fused GRUCell step.

Math (reference):
    xi = x @ [W_ir W_iz W_in] + [b_ir b_iz b_in]
    hh = h @ [W_hr W_hz W_hn]
    r = sigmoid(xr + hr); z = sigmoid(xz + hz)
    n = tanh(xn + r * (hn + b_hn))
    new_h = (1 - z) * n + z * h

Strategy (v2): 2D shard — batch 4-way x hidden 2-way (8 cores). Each core
computes a [B_CORE=4096, H_CORE=512] output tile. Weights per core shrink to
6.3MB (vs 12.6MB replicated), which eliminates the startup DMA wall that
dominated v1 (first matmul at t=52us waiting for all weights to land).

Orientation: the GEMM runs with the *weights* as the stationary operand and
512-wide batch blocks as the moving operand, so each weight byte is loaded
once and amortized over a whole batch block (weight-arrival rate needed at
startup is 8x lower than batch-stationary). Outputs land transposed
([H-partition, batch-free]); h is supplied host-transposed and the final
unshard transposes back. Per-gate biases become per-partition scalars, fused
into the ScalarE activation for free.

fp16 matmul runs at 1 cycle/row on the PE; accumulation is fp32 in PSUM.
DMA issue order is hand-scheduled on the sync queue (one FIFO): first-chunk
weights and the first lhsT block go first, then remaining weights interleaved
ahead of prefetches, so the PE starts ~11us in and never starves afterward.
"""

import os
import sys

import numpy as np

sys.path.insert(0, "/opt/trn_rl_repo")
os.environ.setdefault("MYCRO_LOCAL_CACHE", "1")

import concourse.bass as bass  # noqa: E402
import concourse.mybir as mybir  # noqa: E402
import concourse.tile as tile  # noqa: E402
from concourse import bacc  # noqa: E402
from concourse.bass_utils import run_bass_kernel_spmd  # noqa: E402

N_CORES = 8
B_SHARDS = 4
H_SHARDS = 2
B = 16384
F = 1024  # input feature dim
H = 1024  # hidden dim
K = F + H  # GEMM contraction dim (x features then h features)
P = 128
KO = K // P  # 16 k-chunks of 128
KOX = F // P  # 8 k-chunks belonging to the x part
B_CORE = B // B_SHARDS  # 4096
H_CORE = H // H_SHARDS  # 512
MBLK = 512  # batch columns per moving-operand block (= PSUM bank width)
NBLK = B_CORE // MBLK  # 8
HC_N = H_CORE // P  # 4 hidden-column chunks of 128 (PSUM partition dim)


def build_gru_program(with_bias: bool) -> bass.Bass:
    """One SPMD program; every core runs it on its own (batch, hidden) tile."""
    fp16 = mybir.dt.float16
    f32 = mybir.dt.float32

    # Bacc (not plain Bass): its compile pipeline splits multi-sem waits into
    # event semaphores — walrus rejects >1 wait on most engine instructions.
    nc = bacc.Bacc()
    # Host-prearranged so every DMA lands contiguously per partition:
    # lhsT[b, p, ko, m] = concat(x,h).T[ko*P+p, b*MBLK+m]
    lhsT = nc.declare_dram_parameter("lhsT", [NBLK, P, KO, MBLK], fp16, isOutput=False)
    # w[g, hc, p, ko, n] = Wcat_g[ko*P+p, hc*P+n]  (g: 0=r, 1=z, 2=n)
    w = nc.declare_dram_parameter("w", [3, HC_N, P, KO, P], fp16, isOutput=False)
    # hT[j, m] = h_shard[m, j]  (host-transposed h slice, fp32)
    hT = nc.declare_dram_parameter("hT", [H_CORE, B_CORE], f32, isOutput=False)
    if with_bias:
        # bias[p, g, hc] = b_g[hj*H_CORE + hc*P + p]; g: 0=b_ir 1=b_iz 2=b_in 3=b_hn
        biasp = nc.declare_dram_parameter("bias", [P, 4, HC_N], f32, isOutput=False)
    out = nc.declare_dram_parameter("out", [H_CORE, B_CORE], f32, isOutput=True)

    Sigmoid = mybir.ActivationFunctionType.Sigmoid
    Tanh = mybir.ActivationFunctionType.Tanh

    with tile.TileContext(nc) as tc:
        with (
            tc.tile_pool(name="wpool", bufs=1) as wpool,
            tc.tile_pool(name="lpool", bufs=3) as lpool,
            tc.tile_pool(name="hpool", bufs=2) as hpool,
            tc.tile_pool(name="opool", bufs=3) as opool,
            tc.tile_pool(name="epool", bufs=2) as epool,
            tc.tile_pool(name="psum", bufs=2, space="PSUM") as psum,
        ):
            wsb = {}

            def load_w(g: int, hc: int):
                t = wpool.tile([P, KO, P], fp16, tag=f"w{g}_{hc}")
                nc.sync.dma_start(t[:], w[g, hc])
                wsb[(g, hc)] = t

            def load_lt(b: int):
                # two ko-halves so the first matmuls start after 1MB, not 2MB
                t = lpool.tile([P, KO, MBLK], fp16, tag="lt")
                half = KO // 2
                nc.sync.dma_start(t[:, 0:half, :], lhsT[b, :, 0:half, :])
                nc.sync.dma_start(t[:, half:KO, :], lhsT[b, :, half:KO, :])
                return t

            # --- startup-critical DMA order (sync queue is one FIFO) ---
            load_w(0, 0)
            lt = load_lt(0)
            load_w(1, 0)
            load_w(2, 0)
            bias_sb = None
            if with_bias:
                bias_sb = wpool.tile([P, 4, HC_N], f32, tag="bias_sb")
                nc.sync.dma_start(bias_sb[:], biasp[:])

            lt_next = None
            for b in range(NBLK):
                for hc in range(HC_N):
                    # block 0 pulls in the remaining weights just ahead of use
                    if b == 0 and hc >= 1:
                        for g in range(3):
                            load_w(g, hc)
                    # prefetch next batch block mid-way through this one
                    if hc == 2 and b + 1 < NBLK:
                        lt_next = load_lt(b + 1)

                    ht = hpool.tile([P, MBLK], f32, tag=f"ht{hc}")
                    nc.sync.dma_start(
                        ht[:], hT[hc * P : (hc + 1) * P, b * MBLK : (b + 1) * MBLK]
                    )

                    pr = psum.tile([P, MBLK], f32, tag="pr")
                    pz = psum.tile([P, MBLK], f32, tag="pz")
                    pxn = psum.tile([P, MBLK], f32, tag="pxn")
                    phn = psum.tile([P, MBLK], f32, tag="phn")

                    # gate sweeps: stationary = weight chunk, moving = batch
                    for ko in range(KO):
                        nc.tensor.matmul(
                            pr[:],
                            wsb[(0, hc)][:, ko, :],
                            lt[:, ko, :],
                            start=(ko == 0),
                            stop=(ko == KO - 1),
                        )
                    for ko in range(KO):
                        nc.tensor.matmul(
                            pz[:],
                            wsb[(1, hc)][:, ko, :],
                            lt[:, ko, :],
                            start=(ko == 0),
                            stop=(ko == KO - 1),
                        )
                    for ko in range(KOX):
                        nc.tensor.matmul(
                            pxn[:],
                            wsb[(2, hc)][:, ko, :],
                            lt[:, ko, :],
                            start=(ko == 0),
                            stop=(ko == KOX - 1),
                        )
                    for ko in range(KOX, KO):
                        nc.tensor.matmul(
                            phn[:],
                            wsb[(2, hc)][:, ko, :],
                            lt[:, ko, :],
                            start=(ko == KOX),
                            stop=(ko == KO - 1),
                        )

                    sr = epool.tile([P, MBLK], f32, tag="sr")
                    sz = epool.tile([P, MBLK], f32, tag="sz")
                    sn = epool.tile([P, MBLK], f32, tag="sn")
                    tt = epool.tile([P, MBLK], f32, tag="tt")
                    if with_bias:
                        nc.scalar.activation(
                            sr[:], pr[:], Sigmoid, bias=bias_sb[:, 0, hc]
                        )
                        nc.scalar.activation(
                            sz[:], pz[:], Sigmoid, bias=bias_sb[:, 1, hc]
                        )
                        nc.vector.tensor_scalar_add(tt[:], phn[:], bias_sb[:, 3, hc])
                        nc.vector.tensor_mul(tt[:], sr[:], tt[:])
                        nc.vector.tensor_add(tt[:], tt[:], pxn[:])
                        nc.scalar.activation(sn[:], tt[:], Tanh, bias=bias_sb[:, 2, hc])
                    else:
                        nc.scalar.activation(sr[:], pr[:], Sigmoid)
                        nc.scalar.activation(sz[:], pz[:], Sigmoid)
                        nc.vector.tensor_mul(tt[:], sr[:], phn[:])
                        nc.vector.tensor_add(tt[:], tt[:], pxn[:])
                        nc.scalar.activation(sn[:], tt[:], Tanh)
                    ot = opool.tile([P, MBLK], f32, tag="ot")
                    nc.vector.tensor_sub(tt[:], ht[:], sn[:])
                    nc.vector.tensor_mul(tt[:], tt[:], sz[:])
                    nc.vector.tensor_add(ot[:], sn[:], tt[:])
                    nc.sync.dma_start(
                        out[hc * P : (hc + 1) * P, b * MBLK : (b + 1) * MBLK], ot[:]
                    )
                if lt_next is not None:
                    lt = lt_next
                    lt_next = None
    nc.finalize()
    return nc


_PROGRAM_CACHE: dict = {}


def get_program(with_bias: bool) -> bass.Bass:
    if with_bias not in _PROGRAM_CACHE:
        _PROGRAM_CACHE[with_bias] = build_gru_program(with_bias)
    return _PROGRAM_CACHE[with_bias]


def prepare_in_maps(h, x, W_ir, W_iz, W_in, b_ir, b_iz, b_in, W_hr, W_hz, W_hn, b_hn):
    """Host-side shard + layout prep. Returns (in_maps, with_bias)."""
    h = np.ascontiguousarray(np.asarray(h, dtype=np.float32))
    x = np.ascontiguousarray(np.asarray(x, dtype=np.float32))
    assert x.shape == (B, F) and h.shape == (B, H), (x.shape, h.shape)

    wcat = [
        np.concatenate([W_ir, W_hr], axis=0).astype(np.float16),
        np.concatenate([W_iz, W_hz], axis=0).astype(np.float16),
        np.concatenate([W_in, W_hn], axis=0).astype(np.float16),
    ]  # each [K, H]

    br = np.asarray(b_ir, np.float32)
    bz = np.asarray(b_iz, np.float32)
    bn = np.asarray(b_in, np.float32)
    bhn = np.asarray(b_hn, np.float32)
    biases = np.stack([br, bz, bn, bhn])  # [4, H]
    with_bias = bool(np.any(biases != 0.0))

    # per H-shard: weights in the exact SBUF layout [3, HC_N, P, KO, P]
    w_shards = []
    bias_shards = []
    for hj in range(H_SHARDS):
        cs = slice(hj * H_CORE, (hj + 1) * H_CORE)
        ws = np.empty((3, HC_N, P, KO, P), np.float16)
        for g in range(3):
            # [K, H_CORE] -> [KO, P, HC_N, P] -> [HC_N, P, KO, P]
            wg = wcat[g][:, cs].reshape(KO, P, HC_N, P)
            ws[g] = wg.transpose(2, 1, 0, 3)
        w_shards.append(np.ascontiguousarray(ws))
        if with_bias:
            # [4, H_CORE] -> [4, HC_N, P] -> [P, 4, HC_N]
            bs = biases[:, cs].reshape(4, HC_N, P).transpose(2, 0, 1)
            bias_shards.append(np.ascontiguousarray(bs.astype(np.float32)))

    # per batch-shard: lhsT blocks [NBLK, P, KO, MBLK] and hT slices
    lhsT_shards = []
    hT_shards = []
    for bi in range(B_SHARDS):
        sl = slice(bi * B_CORE, (bi + 1) * B_CORE)
        lhsT_full = np.empty((K, B_CORE), np.float16)
        lhsT_full[:F] = x[sl].T
        lhsT_full[F:] = h[sl].T
        # [K, B_CORE] -> [KO, P, NBLK, MBLK] -> [NBLK, P, KO, MBLK]
        lt = lhsT_full.reshape(KO, P, NBLK, MBLK).transpose(2, 1, 0, 3)
        lhsT_shards.append(np.ascontiguousarray(lt))
        hT_shards.append(np.ascontiguousarray(h[sl].T))  # [H, B_CORE]

    in_maps = []
    for c in range(N_CORES):
        bi, hj = divmod(c, H_SHARDS)
        m = {
            "lhsT": lhsT_shards[bi],
            "w": w_shards[hj],
            "hT": np.ascontiguousarray(
                hT_shards[bi][hj * H_CORE : (hj + 1) * H_CORE]
            ),
        }
        if with_bias:
            m["bias"] = bias_shards[hj]
        in_maps.append(m)
    return in_maps, with_bias


def kernel(h, x, W_ir, W_iz, W_in, b_ir, b_iz, b_in, W_hr, W_hz, W_hn, b_hn):
    in_maps, with_bias = prepare_in_maps(
        h, x, W_ir, W_iz, W_in, b_ir, b_iz, b_in, W_hr, W_hz, W_hn, b_hn
    )
    nc = get_program(with_bias)
    res = run_bass_kernel_spmd(nc, in_maps, list(range(N_CORES)))
    new_h = np.empty((B, H), np.float32)
    for c in range(N_CORES):
        bi, hj = divmod(c, H_SHARDS)
        outT = res.results[c]["out"]  # [H_CORE, B_CORE]
        new_h[bi * B_CORE : (bi + 1) * B_CORE, hj * H_CORE : (hj + 1) * H_CORE] = (
            outT.T
        )
    return (new_h, new_h)


# revision 5
# speedup vs baseline: 1.2669x; 1.1466x over previous
"""Trainium2 Bass kernel for a fused GRUCell step.

Math (reference):
    xi = x @ [W_ir W_iz W_in] + [b_ir b_iz b_in]
    hh = h @ [W_hr W_hz W_hn]
    r = sigmoid(xr + hr); z = sigmoid(xz + hz)
    n = tanh(xn + r * (hn + b_hn))
    new_h = (1 - z) * n + z * h

Strategy (v2): 2D shard — batch 4-way x hidden 2-way (8 cores). Each core
computes a [B_CORE=4096, H_CORE=512] output tile. Weights per core shrink to
6.3MB (vs 12.6MB replicated), which eliminates the startup DMA wall that
dominated v1 (first matmul at t=52us waiting for all weights to land).

Orientation: the GEMM runs with the *weights* as the stationary operand and
512-wide batch blocks as the moving operand, so each weight byte is loaded
once and amortized over a whole batch block (weight-arrival rate needed at
startup is 8x lower than batch-stationary). Outputs land transposed
([H-partition, batch-free]); h is supplied host-transposed and the final
unshard transposes back. Per-gate biases become per-partition scalars, fused
into the ScalarE activation for free.

fp16 matmul runs at 1 cycle/row on the PE; accumulation is fp32 in PSUM.
DMA issue order is hand-scheduled on the sync queue (one FIFO): first-chunk
weights and the first lhsT block go first, then remaining weights interleaved
ahead of prefetches, so the PE starts ~11us in and never starves afterward.
"""

import os
import sys

import numpy as np

sys.path.insert(0, "/opt/trn_rl_repo")
os.environ.setdefault("MYCRO_LOCAL_CACHE", "1")

import concourse.bass as bass  # noqa: E402
import concourse.mybir as mybir  # noqa: E402
import concourse.tile as tile  # noqa: E402
from concourse import bacc  # noqa: E402
from concourse.bass_utils import run_bass_kernel_spmd  # noqa: E402

N_CORES = 8
B_SHARDS = 4
H_SHARDS = 2
B = 16384
F = 1024  # input feature dim
H = 1024  # hidden dim
K = F + H  # GEMM contraction dim (x features then h features)
P = 128
KO = K // P  # 16 k-chunks of 128
KOX = F // P  # 8 k-chunks belonging to the x part
B_CORE = B // B_SHARDS  # 4096
H_CORE = H // H_SHARDS  # 512
MBLK = 512  # batch columns per moving-operand block (= PSUM bank width)
NBLK = B_CORE // MBLK  # 8
HC_N = H_CORE // P  # 4 hidden-column chunks of 128 (PSUM partition dim)
KO8 = K // 256  # 8 k-chunks of 256 for fp8 DoubleRow (r gate)
WS = 16.0  # fp8 weight scale: lifts w std ~0.031 into e4m3 normal range


def build_gru_program(with_bias: bool) -> bass.Bass:
    """One SPMD program; every core runs it on its own (batch, hidden) tile."""
    fp16 = mybir.dt.float16
    f32 = mybir.dt.float32

    # Bacc (not plain Bass): its compile pipeline splits multi-sem waits into
    # event semaphores — walrus rejects >1 wait on most engine instructions.
    nc = bacc.Bacc()
    # Host-prearranged so every DMA lands contiguously per partition:
    # lhsT[b, p, ko, m] = concat(x,h).T[ko*P+p, b*MBLK+m]
    lhsT = nc.declare_dram_parameter("lhsT", [NBLK, P, KO, MBLK], fp16, isOutput=False)
    fp8 = mybir.dt.float8e4
    # w[g, hc, p, ko, n] = Wcat_g[ko*P+p, hc*P+n]  (g: 0=z, 1=n; r is fp8)
    w = nc.declare_dram_parameter("w", [2, HC_N, P, KO, P], fp16, isOutput=False)
    # r gate in fp8 DoubleRow layout: w8r[hc, p, ko8, i, n] = WS*Wcat_r[ko8*256+i*128+p, hc*P+n]
    w8r = nc.declare_dram_parameter("w8r", [HC_N, P, KO8, 2, P], fp8, isOutput=False)
    # fp8 copy of the moving operand for the r gate, DoubleRow-paired
    lhsT8 = nc.declare_dram_parameter(
        "lhsT8", [NBLK, P, KO8, 2, MBLK], fp8, isOutput=False
    )
    # hT[j, m] = h_shard[m, j]  (host-transposed h slice, fp32)
    hT = nc.declare_dram_parameter("hT", [H_CORE, B_CORE], f32, isOutput=False)
    if with_bias:
        # bias[p, g, hc] = b_g[hj*H_CORE + hc*P + p]; g: 0=b_ir 1=b_iz 2=b_in 3=b_hn
        biasp = nc.declare_dram_parameter("bias", [P, 4, HC_N], f32, isOutput=False)
    out = nc.declare_dram_parameter("out", [H_CORE, B_CORE], f32, isOutput=True)

    Sigmoid = mybir.ActivationFunctionType.Sigmoid
    Tanh = mybir.ActivationFunctionType.Tanh

    with tile.TileContext(nc) as tc:
        with (
            tc.tile_pool(name="wpool", bufs=1) as wpool,
            tc.tile_pool(name="lpool", bufs=3) as lpool,
            tc.tile_pool(name="hpool", bufs=2) as hpool,
            tc.tile_pool(name="opool", bufs=3) as opool,
            tc.tile_pool(name="epool", bufs=2) as epool,
            tc.tile_pool(name="psum", bufs=2, space="PSUM") as psum,
        ):
            wsb = {}
            w8sb = {}

            def load_w(g: int, hc: int):
                t = wpool.tile([P, KO, P], fp16, tag=f"w{g}_{hc}")
                nc.sync.dma_start(t[:], w[g, hc])
                wsb[(g, hc)] = t

            def load_w8(hc: int):
                t = wpool.tile([P, KO8, 2, P], fp8, tag=f"w8_{hc}")
                nc.sync.dma_start(t[:], w8r[hc])
                w8sb[hc] = t

            def load_lt8(b: int):
                t = lpool.tile([P, KO8, 2, MBLK], fp8, tag="lt8")
                half = KO8 // 2
                nc.sync.dma_start(t[:, 0:half, :, :], lhsT8[b, :, 0:half, :, :])
                nc.sync.dma_start(t[:, half:KO8, :, :], lhsT8[b, :, half:KO8, :, :])
                return t

            def load_lt(b: int):
                # two ko-halves so the first matmuls start after 1MB, not 2MB
                t = lpool.tile([P, KO, MBLK], fp16, tag="lt")
                half = KO // 2
                nc.sync.dma_start(t[:, 0:half, :], lhsT[b, :, 0:half, :])
                nc.sync.dma_start(t[:, half:KO, :], lhsT[b, :, half:KO, :])
                return t

            # --- startup-critical DMA order (sync queue is one FIFO) ---
            load_w8(0)
            lt8 = load_lt8(0)
            load_w(0, 0)
            lt = load_lt(0)
            load_w(1, 0)
            bias_sb = None
            if with_bias:
                bias_sb = wpool.tile([P, 4, HC_N], f32, tag="bias_sb")
                nc.sync.dma_start(bias_sb[:], biasp[:])

            lt_next = None
            lt8_next = None
            for b in range(NBLK):
                for hc in range(HC_N):
                    # block 0 pulls in the remaining weights just ahead of use
                    if b == 0 and hc >= 1:
                        load_w8(hc)
                        load_w(0, hc)
                        load_w(1, hc)
                    # prefetch next batch block mid-way through this one
                    if hc == 2 and b + 1 < NBLK:
                        lt8_next = load_lt8(b + 1)
                        lt_next = load_lt(b + 1)

                    ht = hpool.tile([P, MBLK], f32, tag=f"ht{hc}")
                    nc.sync.dma_start(
                        ht[:], hT[hc * P : (hc + 1) * P, b * MBLK : (b + 1) * MBLK]
                    )

                    pr = psum.tile([P, MBLK], f32, tag="pr")
                    pz = psum.tile([P, MBLK], f32, tag="pz")
                    pxn = psum.tile([P, MBLK], f32, tag="pxn")
                    phn = psum.tile([P, MBLK], f32, tag="phn")

                    # gate sweeps: stationary = weight chunk, moving = batch
                    # r gate: fp8 DoubleRow, K=256 per matmul, result is WS*(xr+hr)
                    for ko8 in range(KO8):
                        nc.tensor.matmul(
                            pr[:],
                            w8sb[hc][:, ko8, :, :],
                            lt8[:, ko8, :, :],
                            start=(ko8 == 0),
                            stop=(ko8 == KO8 - 1),
                            perf_mode=mybir.MatmulPerfMode.DoubleRow,
                        )
                    for ko in range(KO):
                        nc.tensor.matmul(
                            pz[:],
                            wsb[(0, hc)][:, ko, :],
                            lt[:, ko, :],
                            start=(ko == 0),
                            stop=(ko == KO - 1),
                        )
                    for ko in range(KOX):
                        nc.tensor.matmul(
                            pxn[:],
                            wsb[(1, hc)][:, ko, :],
                            lt[:, ko, :],
                            start=(ko == 0),
                            stop=(ko == KOX - 1),
                        )
                    for ko in range(KOX, KO):
                        nc.tensor.matmul(
                            phn[:],
                            wsb[(1, hc)][:, ko, :],
                            lt[:, ko, :],
                            start=(ko == KOX),
                            stop=(ko == KO - 1),
                        )

                    sr = epool.tile([P, MBLK], f32, tag="sr")
                    sz = epool.tile([P, MBLK], f32, tag="sz")
                    sn = epool.tile([P, MBLK], f32, tag="sn")
                    tt = epool.tile([P, MBLK], f32, tag="tt")
                    if with_bias:
                        nc.scalar.activation(
                            sr[:],
                            pr[:],
                            Sigmoid,
                            bias=bias_sb[:, 0, hc : hc + 1],
                            scale=1.0 / WS,
                        )
                        nc.scalar.activation(
                            sz[:], pz[:], Sigmoid, bias=bias_sb[:, 1, hc : hc + 1]
                        )
                        nc.vector.tensor_scalar_add(tt[:], phn[:], bias_sb[:, 3, hc : hc + 1])
                        nc.vector.tensor_mul(tt[:], sr[:], tt[:])
                        nc.vector.tensor_add(tt[:], tt[:], pxn[:])
                        nc.scalar.activation(sn[:], tt[:], Tanh, bias=bias_sb[:, 2, hc : hc + 1])
                    else:
                        nc.scalar.activation(sr[:], pr[:], Sigmoid, scale=1.0 / WS)
                        nc.scalar.activation(sz[:], pz[:], Sigmoid)
                        nc.vector.tensor_mul(tt[:], sr[:], phn[:])
                        nc.vector.tensor_add(tt[:], tt[:], pxn[:])
                        nc.scalar.activation(sn[:], tt[:], Tanh)
                    ot = opool.tile([P, MBLK], f32, tag="ot")
                    nc.vector.tensor_sub(tt[:], ht[:], sn[:])
                    nc.vector.tensor_mul(tt[:], tt[:], sz[:])
                    nc.vector.tensor_add(ot[:], sn[:], tt[:])
                    nc.sync.dma_start(
                        out[hc * P : (hc + 1) * P, b * MBLK : (b + 1) * MBLK], ot[:]
                    )
                if lt_next is not None:
                    lt = lt_next
                    lt8 = lt8_next
                    lt_next = None
    nc.finalize()
    return nc


_PROGRAM_CACHE: dict = {}


def get_program(with_bias: bool) -> bass.Bass:
    if with_bias not in _PROGRAM_CACHE:
        _PROGRAM_CACHE[with_bias] = build_gru_program(with_bias)
    return _PROGRAM_CACHE[with_bias]


def prepare_in_maps(h, x, W_ir, W_iz, W_in, b_ir, b_iz, b_in, W_hr, W_hz, W_hn, b_hn):
    """Host-side shard + layout prep. Returns (in_maps, with_bias)."""
    h = np.ascontiguousarray(np.asarray(h, dtype=np.float32))
    x = np.ascontiguousarray(np.asarray(x, dtype=np.float32))
    assert x.shape == (B, F) and h.shape == (B, H), (x.shape, h.shape)

    import ml_dtypes

    fp8np = ml_dtypes.float8_e4m3
    wcat = [
        np.concatenate([W_iz, W_hz], axis=0).astype(np.float16),
        np.concatenate([W_in, W_hn], axis=0).astype(np.float16),
    ]  # each [K, H], gates z and n
    wcat_r = np.concatenate([W_ir, W_hr], axis=0).astype(np.float32)  # [K, H]

    br = np.asarray(b_ir, np.float32)
    bz = np.asarray(b_iz, np.float32)
    bn = np.asarray(b_in, np.float32)
    bhn = np.asarray(b_hn, np.float32)
    biases = np.stack([br, bz, bn, bhn])  # [4, H]
    with_bias = bool(np.any(biases != 0.0))

    # per H-shard: weights in the exact SBUF layout
    w_shards = []
    w8_shards = []
    bias_shards = []
    for hj in range(H_SHARDS):
        cs = slice(hj * H_CORE, (hj + 1) * H_CORE)
        ws = np.empty((2, HC_N, P, KO, P), np.float16)
        for g in range(2):
            # [K, H_CORE] -> [KO, P, HC_N, P] -> [HC_N, P, KO, P]
            wg = wcat[g][:, cs].reshape(KO, P, HC_N, P)
            ws[g] = wg.transpose(2, 1, 0, 3)
        w_shards.append(np.ascontiguousarray(ws))
        # r gate fp8 DoubleRow layout: [K, H_CORE] -> [KO8, 2, P, HC_N, P]
        # -> [HC_N, P, KO8, 2, P]
        w8 = (wcat_r[:, cs] * WS).astype(fp8np)
        w8 = w8.reshape(KO8, 2, P, HC_N, P).transpose(3, 2, 0, 1, 4)
        w8_shards.append(np.ascontiguousarray(w8))
        if with_bias:
            # [4, H_CORE] -> [4, HC_N, P] -> [P, 4, HC_N]
            bs = biases[:, cs].reshape(4, HC_N, P).transpose(2, 0, 1)
            bias_shards.append(np.ascontiguousarray(bs.astype(np.float32)))

    # per batch-shard: lhsT blocks [NBLK, P, KO, MBLK], fp8 copy, hT slices
    lhsT_shards = []
    lhsT8_shards = []
    hT_shards = []
    for bi in range(B_SHARDS):
        sl = slice(bi * B_CORE, (bi + 1) * B_CORE)
        lhsT_full = np.empty((K, B_CORE), np.float16)
        lhsT_full[:F] = x[sl].T
        lhsT_full[F:] = h[sl].T
        # [K, B_CORE] -> [KO, P, NBLK, MBLK] -> [NBLK, P, KO, MBLK]
        lt = lhsT_full.reshape(KO, P, NBLK, MBLK).transpose(2, 1, 0, 3)
        lhsT_shards.append(np.ascontiguousarray(lt))
        l8 = np.empty((K, B_CORE), fp8np)
        l8[:F] = x[sl].T.astype(fp8np)
        l8[F:] = h[sl].T.astype(fp8np)
        # [K, B_CORE] -> [KO8, 2, P, NBLK, MBLK] -> [NBLK, P, KO8, 2, MBLK]
        l8 = l8.reshape(KO8, 2, P, NBLK, MBLK).transpose(3, 2, 0, 1, 4)
        lhsT8_shards.append(np.ascontiguousarray(l8))
        hT_shards.append(np.ascontiguousarray(h[sl].T))  # [H, B_CORE]

    in_maps = []
    for c in range(N_CORES):
        bi, hj = divmod(c, H_SHARDS)
        m = {
            "lhsT": lhsT_shards[bi],
            "lhsT8": lhsT8_shards[bi],
            "w": w_shards[hj],
            "w8r": w8_shards[hj],
            "hT": np.ascontiguousarray(
                hT_shards[bi][hj * H_CORE : (hj + 1) * H_CORE]
            ),
        }
        if with_bias:
            m["bias"] = bias_shards[hj]
        in_maps.append(m)
    return in_maps, with_bias


def kernel(h, x, W_ir, W_iz, W_in, b_ir, b_iz, b_in, W_hr, W_hz, W_hn, b_hn):
    in_maps, with_bias = prepare_in_maps(
        h, x, W_ir, W_iz, W_in, b_ir, b_iz, b_in, W_hr, W_hz, W_hn, b_hn
    )
    nc = get_program(with_bias)
    res = run_bass_kernel_spmd(nc, in_maps, list(range(N_CORES)))
    new_h = np.empty((B, H), np.float32)
    for c in range(N_CORES):
        bi, hj = divmod(c, H_SHARDS)
        outT = res.results[c]["out"]  # [H_CORE, B_CORE]
        new_h[bi * B_CORE : (bi + 1) * B_CORE, hj * H_CORE : (hj + 1) * H_CORE] = (
            outT.T
        )
    return (new_h, new_h)


# revision 8
# speedup vs baseline: 1.3833x; 1.0919x over previous
"""Trainium2 Bass kernel for a fused GRUCell step.

Math (reference):
    xi = x @ [W_ir W_iz W_in] + [b_ir b_iz b_in]
    hh = h @ [W_hr W_hz W_hn]
    r = sigmoid(xr + hr); z = sigmoid(xz + hz)
    n = tanh(xn + r * (hn + b_hn))
    new_h = (1 - z) * n + z * h

Strategy (v2): 2D shard — batch 4-way x hidden 2-way (8 cores). Each core
computes a [B_CORE=4096, H_CORE=512] output tile. Weights per core shrink to
6.3MB (vs 12.6MB replicated), which eliminates the startup DMA wall that
dominated v1 (first matmul at t=52us waiting for all weights to land).

Orientation: the GEMM runs with the *weights* as the stationary operand and
512-wide batch blocks as the moving operand, so each weight byte is loaded
once and amortized over a whole batch block (weight-arrival rate needed at
startup is 8x lower than batch-stationary). Outputs land transposed
([H-partition, batch-free]); h is supplied host-transposed and the final
unshard transposes back. Per-gate biases become per-partition scalars, fused
into the ScalarE activation for free.

fp16 matmul runs at 1 cycle/row on the PE; accumulation is fp32 in PSUM.
DMA issue order is hand-scheduled on the sync queue (one FIFO): first-chunk
weights and the first lhsT block go first, then remaining weights interleaved
ahead of prefetches, so the PE starts ~11us in and never starves afterward.
"""

import os
import sys

import numpy as np

sys.path.insert(0, "/opt/trn_rl_repo")
os.environ.setdefault("MYCRO_LOCAL_CACHE", "1")

import concourse.bass as bass  # noqa: E402
import concourse.mybir as mybir  # noqa: E402
import concourse.tile as tile  # noqa: E402
from concourse import bacc  # noqa: E402
from concourse.bass_utils import run_bass_kernel_spmd  # noqa: E402

N_CORES = 8
B_SHARDS = 4
H_SHARDS = 2
B = 16384
F = 1024  # input feature dim
H = 1024  # hidden dim
K = F + H  # GEMM contraction dim (x features then h features)
P = 128
KO = K // P  # 16 k-chunks of 128
KOX = F // P  # 8 k-chunks belonging to the x part
B_CORE = B // B_SHARDS  # 4096
H_CORE = H // H_SHARDS  # 512
MBLK = 512  # batch columns per moving-operand block (= PSUM bank width)
NBLK = B_CORE // MBLK  # 8
HC_N = H_CORE // P  # 4 hidden-column chunks of 128 (PSUM partition dim)
KO8 = K // 256  # 8 k-chunks of 256 for fp8 DoubleRow (r gate)
KO8X = F // 256  # 4 fp8 k-chunks belonging to the x part
KO8H = H // 256  # 4 fp8 k-chunks for the hn part
WS = 16.0  # fp8 weight scale: lifts w std ~0.031 into e4m3 normal range


def build_gru_program(with_bias: bool) -> bass.Bass:
    """One SPMD program; every core runs it on its own (batch, hidden) tile."""
    fp16 = mybir.dt.float16
    f32 = mybir.dt.float32

    # Bacc (not plain Bass): its compile pipeline splits multi-sem waits into
    # event semaphores — walrus rejects >1 wait on most engine instructions.
    nc = bacc.Bacc()
    # Host-prearranged so every DMA lands contiguously per partition:
    # lhsT[b, p, ko, m] = concat(x,h).T[ko*P+p, b*MBLK+m]
    lhsT = nc.declare_dram_parameter("lhsT", [NBLK, P, KO, MBLK], fp16, isOutput=False)
    fp8 = mybir.dt.float8e4
    # z gate (full K) and xn part (first F rows of n gate) stay fp16
    wz = nc.declare_dram_parameter("wz", [HC_N, P, KO, P], fp16, isOutput=False)
    wxn = nc.declare_dram_parameter("wxn", [HC_N, P, KOX, P], fp16, isOutput=False)
    # r gate (full K) + hn part in fp8 DoubleRow layout, combined:
    # j in [0,KO8): WS*Wcat_r[j*256+i*128+p, hc*P+n]
    # j in [KO8, KO8+KO8H): WS*W_hn[(j-KO8)*256+i*128+p, hc*P+n]
    w8rh = nc.declare_dram_parameter(
        "w8rh", [HC_N, P, KO8 + KO8H, 2, P], fp8, isOutput=False
    )
    # fp8 copy of the moving operand for the r gate, DoubleRow-paired
    lhsT8 = nc.declare_dram_parameter(
        "lhsT8", [NBLK, P, KO8, 2, MBLK], fp8, isOutput=False
    )
    # hT[j, m] = h_shard[m, j]  (host-transposed h slice, fp32)
    hT = nc.declare_dram_parameter("hT", [H_CORE, B_CORE], f32, isOutput=False)
    if with_bias:
        # bias[p, g, hc] = b_g[hj*H_CORE + hc*P + p]; g: 0=b_ir 1=b_iz 2=b_in 3=b_hn
        biasp = nc.declare_dram_parameter("bias", [P, 4, HC_N], f32, isOutput=False)
    out = nc.declare_dram_parameter("out", [H_CORE, B_CORE], f32, isOutput=True)

    Sigmoid = mybir.ActivationFunctionType.Sigmoid
    Tanh = mybir.ActivationFunctionType.Tanh

    with tile.TileContext(nc) as tc:
        with (
            tc.tile_pool(name="wpool", bufs=1) as wpool,
            tc.tile_pool(name="lpool", bufs=3) as lpool,
            tc.tile_pool(name="hpool", bufs=2) as hpool,
            tc.tile_pool(name="opool", bufs=3) as opool,
            tc.tile_pool(name="epool", bufs=2) as epool,
            tc.tile_pool(name="psum", bufs=2, space="PSUM") as psum,
        ):
            wsb = {}
            w8sb = {}

            def load_wz(hc: int):
                t = wpool.tile([P, KO, P], fp16, tag=f"wz_{hc}")
                nc.sync.dma_start(t[:], wz[hc])
                wsb[("z", hc)] = t

            def load_wxn(hc: int):
                t = wpool.tile([P, KOX, P], fp16, tag=f"wxn_{hc}")
                nc.sync.dma_start(t[:], wxn[hc])
                wsb[("xn", hc)] = t

            def load_w8(hc: int):
                t = wpool.tile([P, KO8 + KO8H, 2, P], fp8, tag=f"w8_{hc}")
                nc.sync.dma_start(t[:], w8rh[hc])
                w8sb[hc] = t

            def load_lt8(b: int):
                t = lpool.tile([P, KO8, 2, MBLK], fp8, tag="lt8")
                half = KO8 // 2
                nc.sync.dma_start(t[:, 0:half, :, :], lhsT8[b, :, 0:half, :, :])
                nc.sync.dma_start(t[:, half:KO8, :, :], lhsT8[b, :, half:KO8, :, :])
                return t

            def load_lt(b: int):
                # two ko-halves so the first matmuls start after 1MB, not 2MB
                t = lpool.tile([P, KO, MBLK], fp16, tag="lt")
                half = KO // 2
                nc.sync.dma_start(t[:, 0:half, :], lhsT[b, :, 0:half, :])
                nc.sync.dma_start(t[:, half:KO, :], lhsT[b, :, half:KO, :])
                return t

            # --- startup-critical DMA order (sync queue is one FIFO) ---
            load_w8(0)
            lt8 = load_lt8(0)
            load_wz(0)
            lt = load_lt(0)
            load_wxn(0)
            bias_sb = None
            if with_bias:
                bias_sb = wpool.tile([P, 4, HC_N], f32, tag="bias_sb")
                nc.sync.dma_start(bias_sb[:], biasp[:])

            lt_next = None
            lt8_next = None
            for b in range(NBLK):
                for hc in range(HC_N):
                    # block 0 pulls in the remaining weights just ahead of use
                    if b == 0 and hc >= 1:
                        load_w8(hc)
                        load_wz(hc)
                        load_wxn(hc)
                    # prefetch next batch block mid-way through this one
                    if hc == 2 and b + 1 < NBLK:
                        lt8_next = load_lt8(b + 1)
                        lt_next = load_lt(b + 1)

                    ht = hpool.tile([P, MBLK], f32, tag=f"ht{hc}")
                    nc.sync.dma_start(
                        ht[:], hT[hc * P : (hc + 1) * P, b * MBLK : (b + 1) * MBLK]
                    )

                    pr = psum.tile([P, MBLK], f32, tag="pr")
                    pz = psum.tile([P, MBLK], f32, tag="pz")
                    pxn = psum.tile([P, MBLK], f32, tag="pxn")
                    phn = psum.tile([P, MBLK], f32, tag="phn")

                    # gate sweeps: stationary = weight chunk, moving = batch
                    # r gate: fp8 DoubleRow, K=256 per matmul, result is WS*(xr+hr)
                    for ko8 in range(KO8):
                        nc.tensor.matmul(
                            pr[:],
                            w8sb[hc][:, ko8, :, :],
                            lt8[:, ko8, :, :],
                            start=(ko8 == 0),
                            stop=(ko8 == KO8 - 1),
                            perf_mode=mybir.MatmulPerfMode.DoubleRow,
                        )
                    for ko in range(KO):
                        nc.tensor.matmul(
                            pz[:],
                            wsb[("z", hc)][:, ko, :],
                            lt[:, ko, :],
                            start=(ko == 0),
                            stop=(ko == KO - 1),
                        )
                    for ko in range(KOX):
                        nc.tensor.matmul(
                            pxn[:],
                            wsb[("xn", hc)][:, ko, :],
                            lt[:, ko, :],
                            start=(ko == 0),
                            stop=(ko == KOX - 1),
                        )
                    # hn part: fp8 DoubleRow (ends the chunk so it sits next
                    # to the following chunk's DR r-sweep — fewer mode flips)
                    for j in range(KO8H):
                        nc.tensor.matmul(
                            phn[:],
                            w8sb[hc][:, KO8 + j, :, :],
                            lt8[:, KO8X + j, :, :],
                            start=(j == 0),
                            stop=(j == KO8H - 1),
                            perf_mode=mybir.MatmulPerfMode.DoubleRow,
                        )

                    sr = epool.tile([P, MBLK], f32, tag="sr")
                    sz = epool.tile([P, MBLK], f32, tag="sz")
                    sn = epool.tile([P, MBLK], f32, tag="sn")
                    tt = epool.tile([P, MBLK], f32, tag="tt")
                    if with_bias:
                        nc.scalar.activation(
                            sr[:],
                            pr[:],
                            Sigmoid,
                            bias=bias_sb[:, 0, hc : hc + 1],
                            scale=1.0 / WS,
                        )
                        nc.scalar.activation(
                            sz[:], pz[:], Sigmoid, bias=bias_sb[:, 1, hc : hc + 1]
                        )
                        nc.vector.tensor_scalar(
                            tt[:],
                            phn[:],
                            1.0 / WS,
                            bias_sb[:, 3, hc : hc + 1],
                            mybir.AluOpType.mult,
                            mybir.AluOpType.add,
                        )
                        nc.vector.tensor_mul(tt[:], sr[:], tt[:])
                        nc.vector.tensor_add(tt[:], tt[:], pxn[:])
                        nc.scalar.activation(sn[:], tt[:], Tanh, bias=bias_sb[:, 2, hc : hc + 1])
                    else:
                        nc.scalar.activation(sr[:], pr[:], Sigmoid, scale=1.0 / WS)
                        nc.scalar.activation(sz[:], pz[:], Sigmoid)
                        nc.vector.tensor_scalar_mul(tt[:], phn[:], 1.0 / WS)
                        nc.vector.tensor_mul(tt[:], sr[:], tt[:])
                        nc.vector.tensor_add(tt[:], tt[:], pxn[:])
                        nc.scalar.activation(sn[:], tt[:], Tanh)
                    ot = opool.tile([P, MBLK], f32, tag="ot")
                    nc.vector.tensor_sub(tt[:], ht[:], sn[:])
                    nc.vector.tensor_mul(tt[:], tt[:], sz[:])
                    nc.vector.tensor_add(ot[:], sn[:], tt[:])
                    nc.sync.dma_start(
                        out[hc * P : (hc + 1) * P, b * MBLK : (b + 1) * MBLK], ot[:]
                    )
                if lt_next is not None:
                    lt = lt_next
                    lt8 = lt8_next
                    lt_next = None
    nc.finalize()
    return nc


_PROGRAM_CACHE: dict = {}


def get_program(with_bias: bool) -> bass.Bass:
    if with_bias not in _PROGRAM_CACHE:
        _PROGRAM_CACHE[with_bias] = build_gru_program(with_bias)
    return _PROGRAM_CACHE[with_bias]


def prepare_in_maps(h, x, W_ir, W_iz, W_in, b_ir, b_iz, b_in, W_hr, W_hz, W_hn, b_hn):
    """Host-side shard + layout prep. Returns (in_maps, with_bias)."""
    h = np.ascontiguousarray(np.asarray(h, dtype=np.float32))
    x = np.ascontiguousarray(np.asarray(x, dtype=np.float32))
    assert x.shape == (B, F) and h.shape == (B, H), (x.shape, h.shape)

    import ml_dtypes

    fp8np = ml_dtypes.float8_e4m3
    wcat_z = np.concatenate([W_iz, W_hz], axis=0).astype(np.float16)  # [K, H]
    w_xn = np.asarray(W_in, np.float32).astype(np.float16)  # [F, H]
    wcat_r = np.concatenate([W_ir, W_hr], axis=0).astype(np.float32)  # [K, H]
    w_hn = np.asarray(W_hn, np.float32)  # [H, H]

    br = np.asarray(b_ir, np.float32)
    bz = np.asarray(b_iz, np.float32)
    bn = np.asarray(b_in, np.float32)
    bhn = np.asarray(b_hn, np.float32)
    biases = np.stack([br, bz, bn, bhn])  # [4, H]
    with_bias = bool(np.any(biases != 0.0))

    # per H-shard: weights in the exact SBUF layout
    wz_shards = []
    wxn_shards = []
    w8_shards = []
    bias_shards = []
    for hj in range(H_SHARDS):
        cs = slice(hj * H_CORE, (hj + 1) * H_CORE)
        # [K, H_CORE] -> [KO, P, HC_N, P] -> [HC_N, P, KO, P]
        wzs = wcat_z[:, cs].reshape(KO, P, HC_N, P).transpose(2, 1, 0, 3)
        wz_shards.append(np.ascontiguousarray(wzs))
        wxns = w_xn[:, cs].reshape(KOX, P, HC_N, P).transpose(2, 1, 0, 3)
        wxn_shards.append(np.ascontiguousarray(wxns))
        # r gate + hn part, fp8 DoubleRow layout [HC_N, P, KO8+KO8H, 2, P]
        w8 = np.empty((HC_N, P, KO8 + KO8H, 2, P), fp8np)
        w8r_ = (wcat_r[:, cs] * WS).astype(fp8np)
        w8[:, :, :KO8] = w8r_.reshape(KO8, 2, P, HC_N, P).transpose(3, 2, 0, 1, 4)
        w8h_ = (w_hn[:, cs] * WS).astype(fp8np)
        w8[:, :, KO8:] = w8h_.reshape(KO8H, 2, P, HC_N, P).transpose(3, 2, 0, 1, 4)
        w8_shards.append(np.ascontiguousarray(w8))
        if with_bias:
            # [4, H_CORE] -> [4, HC_N, P] -> [P, 4, HC_N]
            bs = biases[:, cs].reshape(4, HC_N, P).transpose(2, 0, 1)
            bias_shards.append(np.ascontiguousarray(bs.astype(np.float32)))

    # per batch-shard: lhsT blocks [NBLK, P, KO, MBLK], fp8 copy, hT slices
    lhsT_shards = []
    lhsT8_shards = []
    hT_shards = []
    for bi in range(B_SHARDS):
        sl = slice(bi * B_CORE, (bi + 1) * B_CORE)
        lhsT_full = np.empty((K, B_CORE), np.float16)
        lhsT_full[:F] = x[sl].T
        lhsT_full[F:] = h[sl].T
        # [K, B_CORE] -> [KO, P, NBLK, MBLK] -> [NBLK, P, KO, MBLK]
        lt = lhsT_full.reshape(KO, P, NBLK, MBLK).transpose(2, 1, 0, 3)
        lhsT_shards.append(np.ascontiguousarray(lt))
        l8 = np.empty((K, B_CORE), fp8np)
        l8[:F] = x[sl].T.astype(fp8np)
        l8[F:] = h[sl].T.astype(fp8np)
        # [K, B_CORE] -> [KO8, 2, P, NBLK, MBLK] -> [NBLK, P, KO8, 2, MBLK]
        l8 = l8.reshape(KO8, 2, P, NBLK, MBLK).transpose(3, 2, 0, 1, 4)
        lhsT8_shards.append(np.ascontiguousarray(l8))
        hT_shards.append(np.ascontiguousarray(h[sl].T))  # [H, B_CORE]

    in_maps = []
    for c in range(N_CORES):
        bi, hj = divmod(c, H_SHARDS)
        m = {
            "lhsT": lhsT_shards[bi],
            "lhsT8": lhsT8_shards[bi],
            "wz": wz_shards[hj],
            "wxn": wxn_shards[hj],
            "w8rh": w8_shards[hj],
            "hT": np.ascontiguousarray(
                hT_shards[bi][hj * H_CORE : (hj + 1) * H_CORE]
            ),
        }
        if with_bias:
            m["bias"] = bias_shards[hj]
        in_maps.append(m)
    return in_maps, with_bias


def kernel(h, x, W_ir, W_iz, W_in, b_ir, b_iz, b_in, W_hr, W_hz, W_hn, b_hn):
    in_maps, with_bias = prepare_in_maps(
        h, x, W_ir, W_iz, W_in, b_ir, b_iz, b_in, W_hr, W_hz, W_hn, b_hn
    )
    nc = get_program(with_bias)
    res = run_bass_kernel_spmd(nc, in_maps, list(range(N_CORES)))
    new_h = np.empty((B, H), np.float32)
    for c in range(N_CORES):
        bi, hj = divmod(c, H_SHARDS)
        outT = res.results[c]["out"]  # [H_CORE, B_CORE]
        new_h[bi * B_CORE : (bi + 1) * B_CORE, hj * H_CORE : (hj + 1) * H_CORE] = (
            outT.T
        )
    return (new_h, new_h)


# revision 10
# speedup vs baseline: 1.3949x; 1.0084x over previous
"""Trainium2 Bass kernel for a fused GRUCell step.

Math (reference):
    xi = x @ [W_ir W_iz W_in] + [b_ir b_iz b_in]
    hh = h @ [W_hr W_hz W_hn]
    r = sigmoid(xr + hr); z = sigmoid(xz + hz)
    n = tanh(xn + r * (hn + b_hn))
    new_h = (1 - z) * n + z * h

Strategy (v2): 2D shard — batch 4-way x hidden 2-way (8 cores). Each core
computes a [B_CORE=4096, H_CORE=512] output tile. Weights per core shrink to
6.3MB (vs 12.6MB replicated), which eliminates the startup DMA wall that
dominated v1 (first matmul at t=52us waiting for all weights to land).

Orientation: the GEMM runs with the *weights* as the stationary operand and
512-wide batch blocks as the moving operand, so each weight byte is loaded
once and amortized over a whole batch block (weight-arrival rate needed at
startup is 8x lower than batch-stationary). Outputs land transposed
([H-partition, batch-free]); h is supplied host-transposed and the final
unshard transposes back. Per-gate biases become per-partition scalars, fused
into the ScalarE activation for free.

fp16 matmul runs at 1 cycle/row on the PE; accumulation is fp32 in PSUM.
DMA issue order is hand-scheduled on the sync queue (one FIFO): first-chunk
weights and the first lhsT block go first, then remaining weights interleaved
ahead of prefetches, so the PE starts ~11us in and never starves afterward.
"""

import os
import sys

import numpy as np

sys.path.insert(0, "/opt/trn_rl_repo")
os.environ.setdefault("MYCRO_LOCAL_CACHE", "1")

import concourse.bass as bass  # noqa: E402
import concourse.mybir as mybir  # noqa: E402
import concourse.tile as tile  # noqa: E402
from concourse import bacc  # noqa: E402
from concourse.bass_utils import run_bass_kernel_spmd  # noqa: E402

N_CORES = 8
B_SHARDS = 4
H_SHARDS = 2
B = 16384
F = 1024  # input feature dim
H = 1024  # hidden dim
K = F + H  # GEMM contraction dim (x features then h features)
P = 128
KO = K // P  # 16 k-chunks of 128
KOX = F // P  # 8 k-chunks belonging to the x part
B_CORE = B // B_SHARDS  # 4096
H_CORE = H // H_SHARDS  # 512
MBLK = 512  # batch columns per moving-operand block (= PSUM bank width)
NBLK = B_CORE // MBLK  # 8
HC_N = H_CORE // P  # 4 hidden-column chunks of 128 (PSUM partition dim)
KO8 = K // 256  # 8 k-chunks of 256 for fp8 DoubleRow (r gate)
KO8X = F // 256  # 4 fp8 k-chunks belonging to the x part
KO8H = H // 256  # 4 fp8 k-chunks for the hn part
WS = 16.0  # fp8 weight scale: lifts w std ~0.031 into e4m3 normal range


def build_gru_program(with_bias: bool) -> bass.Bass:
    """One SPMD program; every core runs it on its own (batch, hidden) tile."""
    fp16 = mybir.dt.float16
    f32 = mybir.dt.float32

    # Bacc (not plain Bass): its compile pipeline splits multi-sem waits into
    # event semaphores — walrus rejects >1 wait on most engine instructions.
    nc = bacc.Bacc()
    # Host-prearranged so every DMA lands contiguously per partition:
    # lhsT[b, p, ko, m] = concat(x,h).T[ko*P+p, b*MBLK+m]
    lhsT = nc.declare_dram_parameter("lhsT", [NBLK, P, KO, MBLK], fp16, isOutput=False)
    fp8 = mybir.dt.float8e4
    # z gate (full K) and xn part (first F rows of n gate) stay fp16
    wz = nc.declare_dram_parameter("wz", [HC_N, P, KO, P], fp16, isOutput=False)
    wxn = nc.declare_dram_parameter("wxn", [HC_N, P, KOX, P], fp16, isOutput=False)
    # r gate (full K) + hn part in fp8 DoubleRow layout, combined:
    # j in [0,KO8): WS*Wcat_r[j*256+i*128+p, hc*P+n]
    # j in [KO8, KO8+KO8H): WS*W_hn[(j-KO8)*256+i*128+p, hc*P+n]
    w8rh = nc.declare_dram_parameter(
        "w8rh", [HC_N, P, KO8 + KO8H, 2, P], fp8, isOutput=False
    )
    # fp8 copy of the moving operand for the r gate, DoubleRow-paired
    lhsT8 = nc.declare_dram_parameter(
        "lhsT8", [NBLK, P, KO8, 2, MBLK], fp8, isOutput=False
    )
    # hT[j, m] = h_shard[m, j]  (host-transposed h slice; fp16 — only feeds
    # the z*h blend, where fp16 rounding of h is ~3e-4 relative)
    hT = nc.declare_dram_parameter("hT", [H_CORE, B_CORE], fp16, isOutput=False)
    if with_bias:
        # bias[p, g, hc] = b_g[hj*H_CORE + hc*P + p]; g: 0=b_ir 1=b_iz 2=b_in 3=b_hn
        biasp = nc.declare_dram_parameter("bias", [P, 4, HC_N], f32, isOutput=False)
    out = nc.declare_dram_parameter("out", [H_CORE, B_CORE], f32, isOutput=True)

    Sigmoid = mybir.ActivationFunctionType.Sigmoid
    Tanh = mybir.ActivationFunctionType.Tanh

    with tile.TileContext(nc) as tc:
        with (
            tc.tile_pool(name="wpool", bufs=1) as wpool,
            tc.tile_pool(name="lpool", bufs=3) as lpool,
            tc.tile_pool(name="hpool", bufs=2) as hpool,
            tc.tile_pool(name="opool", bufs=3) as opool,
            tc.tile_pool(name="epool", bufs=2) as epool,
            tc.tile_pool(name="psum", bufs=2, space="PSUM") as psum,
        ):
            wsb = {}
            w8sb = {}

            def load_wz(hc: int):
                t = wpool.tile([P, KO, P], fp16, tag=f"wz_{hc}")
                nc.sync.dma_start(t[:], wz[hc])
                wsb[("z", hc)] = t

            def load_wxn(hc: int):
                t = wpool.tile([P, KOX, P], fp16, tag=f"wxn_{hc}")
                nc.sync.dma_start(t[:], wxn[hc])
                wsb[("xn", hc)] = t

            def load_w8(hc: int):
                t = wpool.tile([P, KO8 + KO8H, 2, P], fp8, tag=f"w8_{hc}")
                nc.sync.dma_start(t[:], w8rh[hc])
                w8sb[hc] = t

            def load_lt8(b: int):
                t = lpool.tile([P, KO8, 2, MBLK], fp8, tag="lt8")
                half = KO8 // 2
                nc.sync.dma_start(t[:, 0:half, :, :], lhsT8[b, :, 0:half, :, :])
                nc.sync.dma_start(t[:, half:KO8, :, :], lhsT8[b, :, half:KO8, :, :])
                return t

            def load_lt(b: int):
                # two ko-halves so the first matmuls start after 1MB, not 2MB
                t = lpool.tile([P, KO, MBLK], fp16, tag="lt")
                half = KO // 2
                nc.sync.dma_start(t[:, 0:half, :], lhsT[b, :, 0:half, :])
                nc.sync.dma_start(t[:, half:KO, :], lhsT[b, :, half:KO, :])
                return t

            # --- startup-critical DMA order (sync queue is one FIFO) ---
            load_w8(0)
            lt8 = load_lt8(0)
            load_wz(0)
            lt = load_lt(0)
            load_wxn(0)
            bias_sb = None
            if with_bias:
                bias_sb = wpool.tile([P, 4, HC_N], f32, tag="bias_sb")
                nc.sync.dma_start(bias_sb[:], biasp[:])

            lt_next = None
            lt8_next = None
            for b in range(NBLK):
                for hc in range(HC_N):
                    # block 0 pulls in the remaining weights just ahead of use
                    if b == 0 and hc >= 1:
                        load_w8(hc)
                        load_wz(hc)
                        load_wxn(hc)
                    # prefetch next batch block mid-way through this one
                    if hc == 2 and b + 1 < NBLK:
                        lt8_next = load_lt8(b + 1)
                        lt_next = load_lt(b + 1)

                    ht = hpool.tile([P, MBLK], fp16, tag=f"ht{hc}")
                    nc.sync.dma_start(
                        ht[:], hT[hc * P : (hc + 1) * P, b * MBLK : (b + 1) * MBLK]
                    )

                    pr = psum.tile([P, MBLK], f32, tag="pr")
                    pz = psum.tile([P, MBLK], f32, tag="pz")
                    pxn = psum.tile([P, MBLK], f32, tag="pxn")
                    phn = psum.tile([P, MBLK], f32, tag="phn")

                    # gate sweeps: stationary = weight chunk, moving = batch
                    # r gate: fp8 DoubleRow, K=256 per matmul, result is WS*(xr+hr)
                    for ko8 in range(KO8):
                        nc.tensor.matmul(
                            pr[:],
                            w8sb[hc][:, ko8, :, :],
                            lt8[:, ko8, :, :],
                            start=(ko8 == 0),
                            stop=(ko8 == KO8 - 1),
                            perf_mode=mybir.MatmulPerfMode.DoubleRow,
                        )
                    for ko in range(KO):
                        nc.tensor.matmul(
                            pz[:],
                            wsb[("z", hc)][:, ko, :],
                            lt[:, ko, :],
                            start=(ko == 0),
                            stop=(ko == KO - 1),
                        )
                    for ko in range(KOX):
                        nc.tensor.matmul(
                            pxn[:],
                            wsb[("xn", hc)][:, ko, :],
                            lt[:, ko, :],
                            start=(ko == 0),
                            stop=(ko == KOX - 1),
                        )
                    # hn part: fp8 DoubleRow (ends the chunk so it sits next
                    # to the following chunk's DR r-sweep — fewer mode flips)
                    for j in range(KO8H):
                        nc.tensor.matmul(
                            phn[:],
                            w8sb[hc][:, KO8 + j, :, :],
                            lt8[:, KO8X + j, :, :],
                            start=(j == 0),
                            stop=(j == KO8H - 1),
                            perf_mode=mybir.MatmulPerfMode.DoubleRow,
                        )

                    sr = epool.tile([P, MBLK], f32, tag="sr")
                    sz = epool.tile([P, MBLK], f32, tag="sz")
                    sn = epool.tile([P, MBLK], f32, tag="sn")
                    tt = epool.tile([P, MBLK], f32, tag="tt")
                    ot = opool.tile([P, MBLK], f32, tag="ot")

                    def epilogue(lo: int, hi: int):
                        s = slice(lo, hi)
                        if with_bias:
                            nc.scalar.activation(
                                sr[:, s],
                                pr[:, s],
                                Sigmoid,
                                bias=bias_sb[:, 0, hc : hc + 1],
                                scale=1.0 / WS,
                            )
                            nc.scalar.activation(
                                sz[:, s],
                                pz[:, s],
                                Sigmoid,
                                bias=bias_sb[:, 1, hc : hc + 1],
                            )
                            nc.vector.tensor_scalar(
                                tt[:, s],
                                phn[:, s],
                                1.0 / WS,
                                bias_sb[:, 3, hc : hc + 1],
                                mybir.AluOpType.mult,
                                mybir.AluOpType.add,
                            )
                            nc.vector.tensor_mul(tt[:, s], sr[:, s], tt[:, s])
                            nc.vector.tensor_add(tt[:, s], tt[:, s], pxn[:, s])
                            nc.scalar.activation(
                                sn[:, s],
                                tt[:, s],
                                Tanh,
                                bias=bias_sb[:, 2, hc : hc + 1],
                            )
                        else:
                            nc.scalar.activation(
                                sr[:, s], pr[:, s], Sigmoid, scale=1.0 / WS
                            )
                            nc.scalar.activation(sz[:, s], pz[:, s], Sigmoid)
                            nc.vector.tensor_scalar_mul(tt[:, s], phn[:, s], 1.0 / WS)
                            nc.vector.tensor_mul(tt[:, s], sr[:, s], tt[:, s])
                            nc.vector.tensor_add(tt[:, s], tt[:, s], pxn[:, s])
                            nc.scalar.activation(sn[:, s], tt[:, s], Tanh)
                        nc.vector.tensor_sub(tt[:, s], ht[:, s], sn[:, s])
                        nc.vector.tensor_mul(tt[:, s], tt[:, s], sz[:, s])
                        nc.vector.tensor_add(ot[:, s], sn[:, s], tt[:, s])
                        nc.sync.dma_start(
                            out[
                                hc * P : (hc + 1) * P,
                                b * MBLK + lo : b * MBLK + hi,
                            ],
                            ot[:, s],
                        )

                    if b == NBLK - 1 and hc == HC_N - 1:
                        # last chunk: pipeline the epilogue in column pieces so
                        # the post-matmul tail is short
                        for lo in range(0, MBLK, P):
                            epilogue(lo, lo + P)
                    else:
                        epilogue(0, MBLK)
                if lt_next is not None:
                    lt = lt_next
                    lt8 = lt8_next
                    lt_next = None
    nc.finalize()
    return nc


_PROGRAM_CACHE: dict = {}


def get_program(with_bias: bool) -> bass.Bass:
    if with_bias not in _PROGRAM_CACHE:
        _PROGRAM_CACHE[with_bias] = build_gru_program(with_bias)
    return _PROGRAM_CACHE[with_bias]


def prepare_in_maps(h, x, W_ir, W_iz, W_in, b_ir, b_iz, b_in, W_hr, W_hz, W_hn, b_hn):
    """Host-side shard + layout prep. Returns (in_maps, with_bias)."""
    h = np.ascontiguousarray(np.asarray(h, dtype=np.float32))
    x = np.ascontiguousarray(np.asarray(x, dtype=np.float32))
    assert x.shape == (B, F) and h.shape == (B, H), (x.shape, h.shape)

    import ml_dtypes

    fp8np = ml_dtypes.float8_e4m3
    wcat_z = np.concatenate([W_iz, W_hz], axis=0).astype(np.float16)  # [K, H]
    w_xn = np.asarray(W_in, np.float32).astype(np.float16)  # [F, H]
    wcat_r = np.concatenate([W_ir, W_hr], axis=0).astype(np.float32)  # [K, H]
    w_hn = np.asarray(W_hn, np.float32)  # [H, H]

    br = np.asarray(b_ir, np.float32)
    bz = np.asarray(b_iz, np.float32)
    bn = np.asarray(b_in, np.float32)
    bhn = np.asarray(b_hn, np.float32)
    biases = np.stack([br, bz, bn, bhn])  # [4, H]
    with_bias = bool(np.any(biases != 0.0))

    # per H-shard: weights in the exact SBUF layout
    wz_shards = []
    wxn_shards = []
    w8_shards = []
    bias_shards = []
    for hj in range(H_SHARDS):
        cs = slice(hj * H_CORE, (hj + 1) * H_CORE)
        # [K, H_CORE] -> [KO, P, HC_N, P] -> [HC_N, P, KO, P]
        wzs = wcat_z[:, cs].reshape(KO, P, HC_N, P).transpose(2, 1, 0, 3)
        wz_shards.append(np.ascontiguousarray(wzs))
        wxns = w_xn[:, cs].reshape(KOX, P, HC_N, P).transpose(2, 1, 0, 3)
        wxn_shards.append(np.ascontiguousarray(wxns))
        # r gate + hn part, fp8 DoubleRow layout [HC_N, P, KO8+KO8H, 2, P]
        w8 = np.empty((HC_N, P, KO8 + KO8H, 2, P), fp8np)
        w8r_ = (wcat_r[:, cs] * WS).astype(fp8np)
        w8[:, :, :KO8] = w8r_.reshape(KO8, 2, P, HC_N, P).transpose(3, 2, 0, 1, 4)
        w8h_ = (w_hn[:, cs] * WS).astype(fp8np)
        w8[:, :, KO8:] = w8h_.reshape(KO8H, 2, P, HC_N, P).transpose(3, 2, 0, 1, 4)
        w8_shards.append(np.ascontiguousarray(w8))
        if with_bias:
            # [4, H_CORE] -> [4, HC_N, P] -> [P, 4, HC_N]
            bs = biases[:, cs].reshape(4, HC_N, P).transpose(2, 0, 1)
            bias_shards.append(np.ascontiguousarray(bs.astype(np.float32)))

    # per batch-shard: lhsT blocks [NBLK, P, KO, MBLK], fp8 copy, hT slices
    lhsT_shards = []
    lhsT8_shards = []
    hT_shards = []
    for bi in range(B_SHARDS):
        sl = slice(bi * B_CORE, (bi + 1) * B_CORE)
        lhsT_full = np.empty((K, B_CORE), np.float16)
        lhsT_full[:F] = x[sl].T
        lhsT_full[F:] = h[sl].T
        # [K, B_CORE] -> [KO, P, NBLK, MBLK] -> [NBLK, P, KO, MBLK]
        lt = lhsT_full.reshape(KO, P, NBLK, MBLK).transpose(2, 1, 0, 3)
        lhsT_shards.append(np.ascontiguousarray(lt))
        l8 = np.empty((K, B_CORE), fp8np)
        l8[:F] = x[sl].T.astype(fp8np)
        l8[F:] = h[sl].T.astype(fp8np)
        # [K, B_CORE] -> [KO8, 2, P, NBLK, MBLK] -> [NBLK, P, KO8, 2, MBLK]
        l8 = l8.reshape(KO8, 2, P, NBLK, MBLK).transpose(3, 2, 0, 1, 4)
        lhsT8_shards.append(np.ascontiguousarray(l8))
        hT_shards.append(np.ascontiguousarray(h[sl].T.astype(np.float16)))  # [H, B_CORE]

    in_maps = []
    for c in range(N_CORES):
        bi, hj = divmod(c, H_SHARDS)
        m = {
            "lhsT": lhsT_shards[bi],
            "lhsT8": lhsT8_shards[bi],
            "wz": wz_shards[hj],
            "wxn": wxn_shards[hj],
            "w8rh": w8_shards[hj],
            "hT": np.ascontiguousarray(
                hT_shards[bi][hj * H_CORE : (hj + 1) * H_CORE]
            ),
        }
        if with_bias:
            m["bias"] = bias_shards[hj]
        in_maps.append(m)
    return in_maps, with_bias


def kernel(h, x, W_ir, W_iz, W_in, b_ir, b_iz, b_in, W_hr, W_hz, W_hn, b_hn):
    in_maps, with_bias = prepare_in_maps(
        h, x, W_ir, W_iz, W_in, b_ir, b_iz, b_in, W_hr, W_hz, W_hn, b_hn
    )
    nc = get_program(with_bias)
    res = run_bass_kernel_spmd(nc, in_maps, list(range(N_CORES)))
    new_h = np.empty((B, H), np.float32)
    for c in range(N_CORES):
        bi, hj = divmod(c, H_SHARDS)
        outT = res.results[c]["out"]  # [H_CORE, B_CORE]
        new_h[bi * B_CORE : (bi + 1) * B_CORE, hj * H_CORE : (hj + 1) * H_CORE] = (
            outT.T
        )
    return (new_h, new_h)
